# revision 9
# baseline (speedup 1.0000x reference)
"""Decoder attention (QKV proj + KV-cache scatter + full softmax attention + out proj)
on 8 Trainium2 cores.

Sharding: core = (batch b, head-group g).  b = core//2, g = core%2; each core
handles 8 of the 16 heads for one batch element.

Key algorithmic point: softmax + attn@V are invariant to a permutation of the
key axis, so the reference's masked_scatter of new K/V into the cache is
equivalent to attending over concat([k_new, cache_keep]) where cache_keep are
the cache rows NOT in update_idx (complement set, gathered host-side during
sharding).  No on-device scatter is needed.

Device kernel (per core), all layouts chosen so no on-device transpose is
ever needed:
  - QKV:   qkT  (c', n)  = w_qkT.T @ xT      (c' = 8 q-heads*64 then 8 k-heads*64)
           v    (n,  c') = xT.T @ w_vT
  - attn:  scoresT (j, n) = k_eff.T-chunks @ qT ; exp on ACT (scale folded in);
           attn@V with V augmented by a ones-column -> softmax denominator
           accumulates for free in the same PSUM tile (row 64).
  - norm:  reciprocal + gpsimd partition_broadcast + DVE multiply.
  - proj:  outT (c_out, n) = w_projT.T @ attn_catT   (partial; host sums the
           two head-group partials per batch and adds b_proj).

All matmuls use float32r (1 cycle/row on TRN2 for N>=256 vs 4 for fp32).
"""

import sys

for _p in ("/opt/trn_rl_repo",):
    if _p not in sys.path:
        sys.path.insert(0, _p)

import numpy as np

import concourse.bacc as bacc
import concourse.mybir as mybir
import concourse.tile as tile
from concourse import bass_utils

B, NX, NC, C, H = 4, 1024, 2048, 1024, 16
DH = C // H                      # 64
G = 2                            # head groups (tensor-parallel factor)
HPG = H // G                     # 8 heads per group
CG = HPG * DH                    # 512 channels per group
SCALE = DH ** -0.5
N_CORES = 8
F32 = mybir.dt.float32
F32R = mybir.dt.float32r
EXP = mybir.ActivationFunctionType.Exp

# matmul dtypes per stage (float32r = full-rate, ~tf32 accuracy; accumulation
# is always fp32 in PSUM)
DT_QKV = F32R
DT_SCORES = F32R
DT_AV = F32R
DT_PROJ = F32R

_STATE = {}


def _r(ap, dt):
    return ap.bitcast(dt) if dt is not F32 else ap


def _build(reps: int = 1):
    """Build + compile the per-core Bass program.

    reps > 1 wraps the whole computation in an on-device hardware loop --
    used only for timing (amortizes host->device dispatch latency).
    """
    nc = bacc.Bacc("TRN2", target_bir_lowering=False, debug=False)

    xT_d = nc.dram_tensor("xT", [C, NX], F32R, kind="ExternalInput")
    wqkT_d = nc.dram_tensor("wqkT", [C, 2 * CG], F32R, kind="ExternalInput")
    wvT_d = nc.dram_tensor("wvT", [C, CG], F32R, kind="ExternalInput")
    bqk_d = nc.dram_tensor("bqk", [128, 8], F32, kind="ExternalInput")
    bv_d = nc.dram_tensor("bv", [128, CG], F32, kind="ExternalInput")
    kkeepT_d = nc.dram_tensor("kkeepT", [CG, NC - NX], F32R, kind="ExternalInput")
    vkeep_d = nc.dram_tensor("vkeep", [NC - NX, HPG * (DH + 1)], F32R, kind="ExternalInput")
    wprojT_d = nc.dram_tensor("wprojT", [CG, C], F32R, kind="ExternalInput")
    ones_d = nc.dram_tensor("ones8", [128, 8], F32R, kind="ExternalInput")
    outT_d = nc.dram_tensor("outT", [C, NX], F32, kind="ExternalOutput")

    NJ = NC // 128               # 16 j-tiles over the effective kv length
    VW = DH + 1                  # 65: v columns + ones column per head

    with tile.TileContext(nc) as tc:
        with (
            tc.tile_pool(name="persist", bufs=1) as pp,
            tc.tile_pool(name="work", bufs=1) as wp,
            tc.tile_pool(name="wqkc", bufs=6) as wqkp,
            tc.tile_pool(name="attn", bufs=2) as ep,
            tc.tile_pool(name="nrm", bufs=1) as np_pool,
            tc.tile_pool(name="out_sb", bufs=2) as op,
            tc.tile_pool(name="mm_ps", bufs=2, space="PSUM") as mm_ps,
            tc.tile_pool(name="s_ps", bufs=2, space="PSUM") as s_pool,
            tc.tile_pool(name="av_ps", bufs=1, space="PSUM") as av_pool,
        ):
            # ---- persistent tiles ----
            q_t = [pp.tile([128, NX], F32R, tag=f"q{i}", name=f"q{i}") for i in range(4)]
            k_t = [pp.tile([128, NC], F32R, tag=f"k{i}", name=f"k{i}") for i in range(4)]
            v_t = [pp.tile([128, HPG * VW], F32R, tag=f"v{i}", name=f"v{i}") for i in range(NJ)]
            a_t = [pp.tile([128, NX], F32R, tag=f"a{i}", name=f"a{i}") for i in range(4)]
            bqk_t = pp.tile([128, 8], F32, tag="bqk")
            bv_t = pp.tile([128, CG], F32, tag="bv")
            xT_t = [wp.tile([128, NX], F32R, tag=f"x{i}", name=f"x{i}") for i in range(8)]
            wv_t = [wp.tile([128, CG], F32R, tag=f"wv{i}", name=f"wv{i}") for i in range(8)]
            wpr_t = [wp.tile([128, C], F32R, tag=f"wp{i}", name=f"wp{i}") for i in range(4)]

            nc.sync.dma_start(bqk_t[:], bqk_d.ap())
            nc.sync.dma_start(bv_t[:], bv_d.ap())

            def body():
                # ---- input DMAs ----
                for i in range(4):
                    nc.sync.dma_start(k_t[i][:, NX:NC], kkeepT_d[i * 128:(i + 1) * 128, :])
                for j in range(NJ // 2, NJ):
                    r0 = (j - NJ // 2) * 128
                    nc.sync.dma_start(v_t[j][:], vkeep_d[r0:r0 + 128, :])
                for i in range(8):
                    nc.sync.dma_start(xT_t[i][:], xT_d[i * 128:(i + 1) * 128, :])
                    nc.sync.dma_start(wv_t[i][:], wvT_d[i * 128:(i + 1) * 128, :])
                for i in range(4):
                    nc.sync.dma_start(wpr_t[i][:], wprojT_d[i * 128:(i + 1) * 128, :])

                # ---- phase 1: QKV projections ----
                # q/k: out (c' 128, n 512) accumulated over 8 c_in chunks;
                # w_qk column-chunks streamed from DRAM (each used once)
                for m in range(8):
                    wqk_c = [wqkp.tile([128, 128], F32R, tag="wqkc", name=f"wqkc{m}_{kk}")
                             for kk in range(8)]
                    for kk in range(8):
                        nc.sync.dma_start(
                            wqk_c[kk][:], wqkT_d[kk * 128:(kk + 1) * 128, m * 128:(m + 1) * 128]
                        )
                    for cch in range(2):
                        qps = mm_ps.tile([128, 512], F32, tag="mmp")
                        for kk in range(8):
                            nc.tensor.matmul(
                                qps[:],
                                wqk_c[kk][:],
                                xT_t[kk][:, cch * 512:(cch + 1) * 512],
                                start=(kk == 0),
                                stop=(kk == 7),
                            )
                        if m < 4:
                            dest = q_t[m][:, cch * 512:(cch + 1) * 512]
                        else:
                            dest = k_t[m - 4][:, cch * 512:(cch + 1) * 512]
                        nc.vector.tensor_scalar_add(dest, qps[:], bqk_t[:, m:m + 1])

                # v: out (n 128, c' 512) accumulated over 8 c_in chunks
                for m in range(8):
                    vps = mm_ps.tile([128, CG], F32, tag="mmp")
                    for kk in range(8):
                        nc.tensor.matmul(
                            vps[:],
                            xT_t[kk][:, m * 128:(m + 1) * 128],
                            wv_t[kk][:],
                            start=(kk == 0),
                            stop=(kk == 7),
                        )
                    nc.sync.dma_start(
                        v_t[m][:].rearrange("p (h w) -> p h w", w=VW)[:, :, DH],
                        ones_d.ap(),
                    )
                    for h in range(HPG):
                        nc.vector.tensor_add(
                            v_t[m][:, h * VW:h * VW + DH],
                            vps[:, h * DH:(h + 1) * DH],
                            bv_t[:, h * DH:(h + 1) * DH],
                        )

                # ---- phase 2: attention ----
                for h in range(HPG):
                    hp, po = h // 2, 64 * (h % 2)
                    av = av_pool.tile([VW, NX], F32, tag="av")
                    for j in range(NJ):
                        sps = s_pool.tile([128, NX], F32, tag="sps")
                        for cch in range(2):
                            nc.tensor.matmul(
                                sps[:, cch * 512:(cch + 1) * 512],
                                k_t[hp][po:po + 64, j * 128:(j + 1) * 128],
                                q_t[hp][po:po + 64, cch * 512:(cch + 1) * 512],
                                start=True,
                                stop=True,
                            )
                        et = ep.tile([128, NX], F32R, tag="et")
                        nc.scalar.activation(et[:], sps[:], EXP, scale=SCALE)
                        for cch in range(2):
                            nc.tensor.matmul(
                                av[:, cch * 512:(cch + 1) * 512],
                                v_t[j][:, h * VW:(h + 1) * VW],
                                et[:, cch * 512:(cch + 1) * 512],
                                start=(j == 0),
                                stop=(j == NJ - 1),
                            )
                    recip = np_pool.tile([1, NX], F32, tag="recip")
                    nc.vector.reciprocal(recip[:], av[DH:VW, :])
                    rb = np_pool.tile([64, NX], F32, tag="rb")
                    nc.gpsimd.partition_broadcast(rb[:], recip[:])
                    nc.vector.tensor_mul(a_t[hp][po:po + 64, :], av[0:DH, :], rb[:])

                # ---- phase 3: output projection (partial: this head group) ----
                for m in range(8):
                    for cch in range(2):
                        pps = mm_ps.tile([128, 512], F32, tag="mmp")
                        for kk in range(4):
                            nc.tensor.matmul(
                                pps[:],
                                wpr_t[kk][:, m * 128:(m + 1) * 128],
                                a_t[kk][:, cch * 512:(cch + 1) * 512],
                                start=(kk == 0),
                                stop=(kk == 3),
                            )
                        ot = op.tile([128, 512], F32, tag="ot")
                        nc.vector.tensor_copy(ot[:], pps[:])
                        nc.sync.dma_start(
                            outT_d[m * 128:(m + 1) * 128, cch * 512:(cch + 1) * 512],
                            ot[:],
                        )

            if reps == 1:
                body()
            else:
                with tc.For_i(0, reps, 1):
                    body()

    nc.compile()
    return nc


def _get_nc():
    if "nc" not in _STATE:
        _STATE["nc"] = _build()
    return _STATE["nc"]


def _prep_in_maps(x, update_idx, cache_k, cache_v, w_qkv, b_qkv):
    """Host-side sharding: build the 8 per-core input dicts."""
    x = np.asarray(x, np.float32)
    update_idx = np.asarray(update_idx)
    cache_k = np.asarray(cache_k, np.float32)
    cache_v = np.asarray(cache_v, np.float32)
    w_qkv = np.asarray(w_qkv, np.float32)
    b_qkv = np.asarray(b_qkv, np.float32)

    per_g = []
    for g in range(G):
        qs = slice(g * CG, (g + 1) * CG)
        ks = slice(C + g * CG, C + (g + 1) * CG)
        vs = slice(2 * C + g * CG, 2 * C + (g + 1) * CG)
        wqkT = np.ascontiguousarray(
            np.concatenate([w_qkv[qs], w_qkv[ks]], 0).T
        )                                                    # (C, 2CG)
        wvT = np.ascontiguousarray(w_qkv[vs].T)              # (C, CG)
        bqk = np.ascontiguousarray(
            np.concatenate([b_qkv[qs], b_qkv[ks]]).reshape(8, 128).T
        )                                                    # (128, 8)
        bv = np.broadcast_to(b_qkv[vs][None, :], (128, CG)).copy()
        per_g.append((wqkT, wvT, bqk, bv))

    in_maps = []
    for b in range(B):
        idx = update_idx[b]
        mask = np.ones(NC, bool)
        mask[idx] = False
        keep = np.nonzero(mask)[0]                           # (NC-NX,) sorted
        xT = np.ascontiguousarray(x[b].T)                    # (C, NX)
        for g in range(G):
            wqkT, wvT, bqk, bv = per_g[g]
            hsel = slice(g * HPG, (g + 1) * HPG)
            kk = cache_k[b, hsel][:, keep, :]                # (HPG, NC-NX, DH)
            kkeepT = np.ascontiguousarray(
                kk.transpose(0, 2, 1).reshape(HPG * DH, NC - NX)
            )
            vk = cache_v[b, hsel][:, keep, :].transpose(1, 0, 2)  # (NC-NX, HPG, DH)
            vkeep = np.ascontiguousarray(
                np.concatenate(
                    [vk, np.ones((NC - NX, HPG, 1), np.float32)], axis=2
                ).reshape(NC - NX, HPG * (DH + 1))
            )
            wprojT = _STATE["wprojT"][g]
            in_maps.append(
                dict(
                    xT=xT, wqkT=wqkT, wvT=wvT, bqk=bqk, bv=bv,
                    kkeepT=kkeepT, vkeep=vkeep, wprojT=wprojT,
                    ones8=np.ones((128, 8), np.float32),
                )
            )
    return in_maps


def kernel(x, update_idx, cache_k, cache_v, w_qkv, b_qkv, w_proj, b_proj):
    nc = _get_nc()
    w_proj = np.asarray(w_proj, np.float32)
    b_proj = np.asarray(b_proj, np.float32)
    _STATE["wprojT"] = [
        np.ascontiguousarray(w_proj[:, g * CG:(g + 1) * CG].T) for g in range(G)
    ]
    in_maps = _prep_in_maps(x, update_idx, cache_k, cache_v, w_qkv, b_qkv)
    res = bass_utils.run_bass_kernel_spmd(nc, in_maps, core_ids=list(range(N_CORES)))
    _STATE["last_results"] = res
    out = np.empty((B, NX, C), np.float32)
    for b in range(B):
        acc = res.results[2 * b]["outT"] + res.results[2 * b + 1]["outT"]
        out[b] = acc.T + b_proj
    return out


# revision 15
# speedup vs baseline: 1.4429x; 1.4429x over previous
"""Decoder attention (QKV proj + KV-cache scatter + full softmax attention + out proj)
on 8 Trainium2 cores.

Sharding: core = (batch b, head-group g).  b = core//2, g = core%2; each core
handles 8 of the 16 heads for one batch element.

Key algorithmic point: softmax + attn@V are invariant to a permutation of the
key axis, so the reference's masked_scatter of new K/V into the cache is
equivalent to attending over concat([k_new, cache_keep]) where cache_keep are
the cache rows NOT in update_idx (complement set, gathered host-side during
sharding).  No on-device scatter is needed.

Device kernel (per core), all layouts chosen so no on-device transpose is
ever needed:
  - QKV:   qkT  (c', n)  = w_qkT.T @ xT      (c' = 8 q-heads*64 then 8 k-heads*64)
           v    (n,  c') = xT.T @ w_vT
  - attn:  scoresT (j, n) = k_eff.T-chunks @ qT ; exp on ACT (scale folded in);
           attn@V with V augmented by a ones-column -> softmax denominator
           accumulates for free in the same PSUM tile (row 64).
  - norm:  reciprocal + gpsimd partition_broadcast + DVE multiply.
  - proj:  outT (c_out, n) = w_projT.T @ attn_catT   (partial; host sums the
           two head-group partials per batch and adds b_proj).

All matmuls use float32r (1 cycle/row on TRN2 for N>=256 vs 4 for fp32).
"""

import sys

for _p in ("/opt/trn_rl_repo",):
    if _p not in sys.path:
        sys.path.insert(0, _p)

import numpy as np

import concourse.bacc as bacc
import concourse.mybir as mybir
import concourse.tile as tile
from concourse import bass_utils

B, NX, NC, C, H = 4, 1024, 2048, 1024, 16
DH = C // H                      # 64
G = 2                            # head groups (tensor-parallel factor)
HPG = H // G                     # 8 heads per group
CG = HPG * DH                    # 512 channels per group
SCALE = DH ** -0.5
N_CORES = 8
F32 = mybir.dt.float32
F32R = mybir.dt.float32r
EXP = mybir.ActivationFunctionType.Exp

# matmul dtypes per stage (float32r = full-rate, ~tf32 accuracy; accumulation
# is always fp32 in PSUM)
DT_QKV = F32R
DT_SCORES = F32R
DT_AV = F32R
DT_PROJ = F32R

_STATE = {}


def _r(ap, dt):
    return ap.bitcast(dt) if dt is not F32 else ap


def _build(reps: int = 1):
    """Build + compile the per-core Bass program.

    reps > 1 wraps the whole computation in an on-device hardware loop --
    used only for timing (amortizes host->device dispatch latency).
    """
    nc = bacc.Bacc("TRN2", target_bir_lowering=False, debug=False)

    xT_d = nc.dram_tensor("xT", [C, NX], F32R, kind="ExternalInput")
    wqkT_d = nc.dram_tensor("wqkT", [C, 2 * CG], F32R, kind="ExternalInput")
    wvT_d = nc.dram_tensor("wvT", [C, CG], F32R, kind="ExternalInput")
    bqk_d = nc.dram_tensor("bqk", [128, 8], F32, kind="ExternalInput")
    bv_d = nc.dram_tensor("bv", [128, CG], F32, kind="ExternalInput")
    kkeepT_d = nc.dram_tensor("kkeepT", [CG, NC - NX], F32R, kind="ExternalInput")
    vkeep_d = nc.dram_tensor("vkeep", [NC - NX, HPG * (DH + 1)], F32R, kind="ExternalInput")
    wprojT_d = nc.dram_tensor("wprojT", [CG, C], F32R, kind="ExternalInput")
    ones_d = nc.dram_tensor("ones8", [128, 8], F32R, kind="ExternalInput")
    outT_d = nc.dram_tensor("outT", [C, NX], F32, kind="ExternalOutput")

    NJ = NC // 128               # 16 j-tiles over the effective kv length
    VW = DH + 1                  # 65: v columns + ones column per head

    with tile.TileContext(nc) as tc:
        with (
            tc.tile_pool(name="persist", bufs=1) as pp,
            tc.tile_pool(name="work", bufs=1) as wp,
            tc.tile_pool(name="wqkc", bufs=2) as wqkp,
            tc.tile_pool(name="attn", bufs=3) as ep,
            tc.tile_pool(name="nrm", bufs=2) as np_pool,
            tc.tile_pool(name="out_sb", bufs=1) as op,
            tc.tile_pool(name="ps", bufs=1, space="PSUM") as psp,
        ):
            # ---- persistent tiles ----
            q_t = [pp.tile([128, NX], F32R, tag=f"q{i}", name=f"q{i}") for i in range(4)]
            k_t = [pp.tile([128, NC], F32R, tag=f"k{i}", name=f"k{i}") for i in range(4)]
            v_t = [pp.tile([128, HPG * VW], F32R, tag=f"v{i}", name=f"v{i}") for i in range(NJ)]
            a_t = [pp.tile([128, NX], F32R, tag=f"a{i}", name=f"a{i}") for i in range(4)]
            bqk_t = pp.tile([128, 8], F32, tag="bqk")
            bv_t = pp.tile([128, CG], F32, tag="bv")
            xT_t = [wp.tile([128, NX], F32R, tag=f"x{i}", name=f"x{i}") for i in range(8)]
            wv_t = [wp.tile([128, CG], F32R, tag=f"wv{i}", name=f"wv{i}") for i in range(8)]
            wpr_t = [wp.tile([128, C], F32R, tag=f"wp{i}", name=f"wp{i}") for i in range(4)]

            nc.sync.dma_start(bqk_t[:], bqk_d.ap())
            nc.sync.dma_start(bv_t[:], bv_d.ap())

            def body():
                # ---- priority DMAs, ordered by first use ----
                nc.sync.dma_start(xT_t[0][:], xT_d[0:128, :])
                mqk_t = {}

                def emit_mqk_dma(m):
                    # one strided DMA brings all 8 (c_in x c') chunks for this
                    # m: SBUF[p, kk*128+c] = wqkT[kk*128+p, m*128+c]
                    mqk = wqkp.tile([128, NX], F32R, tag="mqk", name=f"mqk{m}")
                    mqk_t[m] = mqk
                    nc.sync.dma_start(
                        mqk[:].rearrange("p (kk c) -> p kk c", c=128),
                        wqkT_d[:, m * 128:(m + 1) * 128]
                        .rearrange("(kk p) c -> p kk c", p=128),
                    )

                emit_mqk_dma(0)
                for i in range(1, 8):
                    nc.sync.dma_start(xT_t[i][:], xT_d[i * 128:(i + 1) * 128, :])
                emit_mqk_dma(4)
                for i in range(8):
                    nc.sync.dma_start(wv_t[i][:], wvT_d[i * 128:(i + 1) * 128, :])
                for i in range(4):
                    nc.sync.dma_start(k_t[i][:, NX:NC], kkeepT_d[i * 128:(i + 1) * 128, :])
                for j in range(NJ // 2, NJ):
                    r0 = (j - NJ // 2) * 128
                    nc.sync.dma_start(v_t[j][:], vkeep_d[r0:r0 + 128, :])

                def qk_thunks(i):
                    """Matmul/bias thunks for q m-tile i and k m-tile 4+i,
                    drained one per attention j-step."""
                    for m in (i, 4 + i):
                        mqk = mqk_t[m]
                        qps = psp.tile([128, NX], F32, tag="qps", bufs=1, name=f"qps{m}")
                        for kk in range(8):
                            def mm(m=m, kk=kk, qps=qps, mqk=mqk):
                                for cch in range(2):
                                    nc.tensor.matmul(
                                        qps[:, cch * 512:(cch + 1) * 512],
                                        mqk[:, kk * 128:(kk + 1) * 128],
                                        xT_t[kk][:, cch * 512:(cch + 1) * 512],
                                        start=(kk == 0),
                                        stop=(kk == 7),
                                    )
                            yield mm
                        def bias(m=m, qps=qps):
                            if m < 4:
                                dest = q_t[m][:]
                            else:
                                dest = k_t[m - 4][:, 0:NX]
                            nc.vector.tensor_scalar_add(dest, qps[:], bqk_t[:, m:m + 1])
                        yield bias

                pending = []

                def drain(n):
                    for _ in range(n):
                        if not pending:
                            return
                        pending.pop(0)()

                # qk pair 0 runs up front (attention depends on it)
                for th in qk_thunks(0):
                    th()

                # v: out (n 128, c' 512) accumulated over 8 c_in chunks
                for m in range(8):
                    vps = psp.tile([128, NX], F32, tag="sps", bufs=2, name=f"vps{m}")
                    for kk in range(8):
                        nc.tensor.matmul(
                            vps[:, 0:CG],
                            xT_t[kk][:, m * 128:(m + 1) * 128],
                            wv_t[kk][:],
                            start=(kk == 0),
                            stop=(kk == 7),
                        )
                    nc.sync.dma_start(
                        v_t[m][:].rearrange("p (h w) -> p h w", w=VW)[:, :, DH],
                        ones_d.ap(),
                    )
                    for h in range(HPG):
                        nc.vector.tensor_add(
                            v_t[m][:, h * VW:h * VW + DH],
                            vps[:, h * DH:(h + 1) * DH],
                            bv_t[:, h * DH:(h + 1) * DH],
                        )

                for i in range(4):
                    nc.sync.dma_start(wpr_t[i][:], wprojT_d[i * 128:(i + 1) * 128, :])

                # ---- phase 2: attention; j loop software-pipelined (av for
                # j-1 after scores/exp for j) with leftover QKV matmuls
                # drained one per j-step to fill PE idle time ----
                def attn_head(h):
                    hp, po = h // 2, 64 * (h % 2)
                    av = psp.tile([VW, NX], F32, tag="av", bufs=1, name=f"av{h}")
                    ets = [None] * NJ

                    def emit_av(j):
                        for cch in range(2):
                            nc.tensor.matmul(
                                av[:, cch * 512:(cch + 1) * 512],
                                v_t[j][:, h * VW:(h + 1) * VW],
                                ets[j][:, cch * 512:(cch + 1) * 512],
                                start=(j == 0),
                                stop=(j == NJ - 1),
                            )

                    for j in range(NJ):
                        sps = psp.tile([128, NX], F32, tag="sps", bufs=2, name=f"sps{h}_{j}")
                        for cch in range(2):
                            nc.tensor.matmul(
                                sps[:, cch * 512:(cch + 1) * 512],
                                k_t[hp][po:po + 64, j * 128:(j + 1) * 128],
                                q_t[hp][po:po + 64, cch * 512:(cch + 1) * 512],
                                start=True,
                                stop=True,
                            )
                        et = ep.tile([128, NX], F32R, tag="et", name=f"et{h}_{j}")
                        ets[j] = et
                        nc.scalar.activation(et[:], sps[:], EXP, scale=SCALE)
                        drain(1)
                        if j > 0:
                            emit_av(j - 1)
                    emit_av(NJ - 1)
                    recip = np_pool.tile([1, NX], F32, tag="recip")
                    nc.vector.reciprocal(recip[:], av[DH:VW, :])
                    rb = np_pool.tile([64, NX], F32, tag="rb")
                    nc.gpsimd.partition_broadcast(rb[:], recip[:])
                    nc.vector.tensor_mul(a_t[hp][po:po + 64, :], av[0:DH, :], rb[:])

                for h in range(HPG):
                    if h in (0, 2, 4) and h // 2 + 1 < 4:
                        drain(64)            # finish any leftovers first
                        i = h // 2 + 1
                        emit_mqk_dma(i)
                        emit_mqk_dma(4 + i)
                        pending.extend(qk_thunks(i))
                    attn_head(h)
                drain(64)

                # ---- phase 3: output projection (partial: this head group) ----
                for m in range(8):
                    pps = psp.tile([128, NX], F32, tag="sps", bufs=2, name=f"pps{m}")
                    for cch in range(2):
                        for kk in range(4):
                            nc.tensor.matmul(
                                pps[:, cch * 512:(cch + 1) * 512],
                                wpr_t[kk][:, m * 128:(m + 1) * 128],
                                a_t[kk][:, cch * 512:(cch + 1) * 512],
                                start=(kk == 0),
                                stop=(kk == 3),
                            )
                    ot = op.tile([128, NX], F32, tag="ot")
                    nc.vector.tensor_copy(ot[:], pps[:])
                    nc.sync.dma_start(outT_d[m * 128:(m + 1) * 128, :], ot[:])

            if reps == 1:
                body()
            else:
                with tc.For_i(0, reps, 1):
                    body()

    nc.compile()
    return nc


def _get_nc():
    if "nc" not in _STATE:
        _STATE["nc"] = _build()
    return _STATE["nc"]


def _prep_in_maps(x, update_idx, cache_k, cache_v, w_qkv, b_qkv):
    """Host-side sharding: build the 8 per-core input dicts."""
    x = np.asarray(x, np.float32)
    update_idx = np.asarray(update_idx)
    cache_k = np.asarray(cache_k, np.float32)
    cache_v = np.asarray(cache_v, np.float32)
    w_qkv = np.asarray(w_qkv, np.float32)
    b_qkv = np.asarray(b_qkv, np.float32)

    per_g = []
    for g in range(G):
        qs = slice(g * CG, (g + 1) * CG)
        ks = slice(C + g * CG, C + (g + 1) * CG)
        vs = slice(2 * C + g * CG, 2 * C + (g + 1) * CG)
        wqkT = np.ascontiguousarray(
            np.concatenate([w_qkv[qs], w_qkv[ks]], 0).T
        )                                                    # (C, 2CG)
        wvT = np.ascontiguousarray(w_qkv[vs].T)              # (C, CG)
        bqk = np.ascontiguousarray(
            np.concatenate([b_qkv[qs], b_qkv[ks]]).reshape(8, 128).T
        )                                                    # (128, 8)
        bv = np.broadcast_to(b_qkv[vs][None, :], (128, CG)).copy()
        per_g.append((wqkT, wvT, bqk, bv))

    in_maps = []
    for b in range(B):
        idx = update_idx[b]
        mask = np.ones(NC, bool)
        mask[idx] = False
        keep = np.nonzero(mask)[0]                           # (NC-NX,) sorted
        xT = np.ascontiguousarray(x[b].T)                    # (C, NX)
        for g in range(G):
            wqkT, wvT, bqk, bv = per_g[g]
            hsel = slice(g * HPG, (g + 1) * HPG)
            kk = cache_k[b, hsel][:, keep, :]                # (HPG, NC-NX, DH)
            kkeepT = np.ascontiguousarray(
                kk.transpose(0, 2, 1).reshape(HPG * DH, NC - NX)
            )
            vk = cache_v[b, hsel][:, keep, :].transpose(1, 0, 2)  # (NC-NX, HPG, DH)
            vkeep = np.ascontiguousarray(
                np.concatenate(
                    [vk, np.ones((NC - NX, HPG, 1), np.float32)], axis=2
                ).reshape(NC - NX, HPG * (DH + 1))
            )
            wprojT = _STATE["wprojT"][g]
            in_maps.append(
                dict(
                    xT=xT, wqkT=wqkT, wvT=wvT, bqk=bqk, bv=bv,
                    kkeepT=kkeepT, vkeep=vkeep, wprojT=wprojT,
                    ones8=np.ones((128, 8), np.float32),
                )
            )
    return in_maps


def kernel(x, update_idx, cache_k, cache_v, w_qkv, b_qkv, w_proj, b_proj):
    nc = _get_nc()
    w_proj = np.asarray(w_proj, np.float32)
    b_proj = np.asarray(b_proj, np.float32)
    _STATE["wprojT"] = [
        np.ascontiguousarray(w_proj[:, g * CG:(g + 1) * CG].T) for g in range(G)
    ]
    in_maps = _prep_in_maps(x, update_idx, cache_k, cache_v, w_qkv, b_qkv)
    res = bass_utils.run_bass_kernel_spmd(nc, in_maps, core_ids=list(range(N_CORES)))
    _STATE["last_results"] = res
    out = np.empty((B, NX, C), np.float32)
    for b in range(B):
        acc = res.results[2 * b]["outT"] + res.results[2 * b + 1]["outT"]
        out[b] = acc.T + b_proj
    return out
